# revision 47
# baseline (speedup 1.0000x reference)
"""Trainium2 Bass kernel for causal MHA (b=2, n=4096, d_model=768, 12 heads).

Sharding: 8 cores = 2 batches x 4 head-groups (3 heads each).
Each core:
  - receives its batch's Q/K/V pre-transposed ([768, n], d_model on rows)
    plus its head-group's weight slices (also pre-transposed on host).
  - projects qT/kT ([64, n] per head, head dim on partitions) and
    v ([n, 64] per head, tokens on partitions) on-chip.
  - computes scoresT[k, q] = kT^T @ qT for key-block PAIRS into a 2-bank
    PSUM tile, exponentiates both blocks with ONE wide instruction,
    masks the causal boundary blocks, and accumulates
    outT_aug[65, q] += [v | ones]^T @ P in PSUM (row 64 = denominator).
  - normalizes via reciprocal_approx_fast + gpsimd partition_broadcast.
  - applies the output projection with its w_o row-slice; host sums the
    4 bf16 partial outputs per batch (row-parallel linear unshard).

The Scalar (ACT) engine is the bottleneck: ~25M exp elements at
1 elem/cycle/partition/1.2GHz = 174us floor.  This version offloads a
tunable fraction of h2's exps to the Vector engine with a SINGLE-op
int16 Schraudolph: int16(round(2^7/ln2 * x + B)) bit-pattern IS the
bf16 of ~exp(x), so one tensor_scalar produces matmul-ready bf16 P
tiles (the old 2-op int32 variant was slower than the ACT it replaced).
Causal-boundary mask multiplies run on the otherwise-idle GpSimd
engine, and the normalize pipeline reads the PSUM accumulators
directly (no staging copy), keeping Vector lean enough to absorb the
offloaded exps.

Emission order software-pipelines everything: a global pending-AV queue
(depth 3) runs attention score/exp groups ahead of their AV matmuls
ACROSS chunk boundaries; the prologue orders chunk 0's projections
(q01, k01 first) so the first exp issues after ~14 matmuls; the tail
splits the last chunk's normalize in half so the final output
projections overlap the last AV pairs.

Weight-column host layout packs the six 64-wide q/k heads into three full
128-row M-blocks ([q0;q1], [q2;k2], [k0;k1]); k2/q2 are then DMA-copied to
a fourth block so every head's scores matmul sees its qT and kT at the
same partition base (a matmul constraint), with h2 replicated at both
bases so it can alternate row-groups per key block and pair with itself.
"""

import sys

for _p in ("/opt/trn_rl_repo",):
    if _p not in sys.path:
        sys.path.insert(0, _p)

import numpy as np
import ml_dtypes

import concourse.bass as bass  # noqa: F401  (registers engine classes)
import concourse.tile as tile
from concourse import bacc, mybir
import concourse.bass_utils as bass_utils

P = 128
D_MODEL = 768
KO = D_MODEL // P  # 6 contraction chunks of 128
N_HEADS = 12
D_K = 64
N_CORES = 8
H_LOCAL = 3  # heads per core
D_LOCAL = H_LOCAL * D_K  # 192
B = 2
N_TOKENS = 4096
NQ = 512  # query-chunk size (one PSUM bank of fp32)
NT = 512  # token chunk for q/k projection

F32 = mybir.dt.float32
BF16 = mybir.dt.bfloat16
F32R = mybir.dt.float32r
I16 = mybir.dt.int16

# int16 Schraudolph: int16(A*x + B) bit pattern, viewed as bf16, is
# ~exp(x) with ~3.4% max relative error (the shared-denominator softmax
# cancels most of it).  B is 127*2^7 minus an offset tuned numerically;
# -5.25 hedges between truncating and rounding float->int conversion.
FEXP16_A = float(2 ** 7 / np.log(2))
FEXP16_B = float(127.0 * 2 ** 7) - 5.25


def _mm(ap, flavor):
    """View an fp32 AP as the matmul input dtype."""
    if flavor == "f32r":
        return ap.bitcast(F32R)
    return ap


def build_nc(n=N_TOKENS, mm="bf16", dt_x=BF16, dt_pt=BF16, dt_acc=BF16,
             dve8=5, dve_min_j=1, depth=4, op_back=3, op_batch=4):
    # dve8: of every 8 pair-groups, how many route h2's exp to the
    #   Vector engine (int16 Schraudolph) instead of the Scalar engine.
    # dve_min_j: first query chunk eligible for the offload.
    # depth: pending-AV queue depth (score/exp groups emitted ahead of AV).
    # op_back/op_batch: output projections interleave into the last
    #   `op_back` chunks, `op_batch` at a time.
    assert n % NQ == 0 and n % NT == 0 and n % P == 0
    nc = bacc.Bacc("TRN2", target_bir_lowering=False, debug=False,
                   num_devices=N_CORES)

    qt_d = nc.dram_tensor("qt", [D_MODEL, n], dt_x, kind="ExternalInput")
    kt_d = nc.dram_tensor("kt", [D_MODEL, n], dt_x, kind="ExternalInput")
    vt_d = nc.dram_tensor("vt", [D_MODEL, n], dt_x, kind="ExternalInput")
    wqkv_d = nc.dram_tensor("wqkv", [D_MODEL, 3 * D_LOCAL], dt_x,
                            kind="ExternalInput")
    wo_d = nc.dram_tensor("wo", [P, 2 * D_MODEL], dt_x,
                          kind="ExternalInput")
    cm_d = nc.dram_tensor("cm", [P, P], dt_x, kind="ExternalInput")
    y_d = nc.dram_tensor("y", [n, D_MODEL], dt_acc, kind="ExternalOutput")

    qt_r = qt_d.ap().rearrange("(ko ki) t -> ki ko t", ki=P)
    kt_r = kt_d.ap().rearrange("(ko ki) t -> ki ko t", ki=P)
    vt_r = vt_d.ap().rearrange("(ko ki) t -> ki ko t", ki=P)
    wqkv_r = wqkv_d.ap().rearrange("(ko ki) m -> ki ko m", ki=P)

    TCH = n // NT       # q/k projection token chunks
    TB = n // P         # 128-token blocks
    QCH = n // NQ       # query chunks
    KB_PER_Q = NQ // P  # key blocks per query chunk (4)
    NOC = 2             # output-projection column chunks
    NO = D_MODEL // NOC

    # Host weight-column order: [q0 q1 | q2 k2 | k0 k1] -> 3 full M-blocks.
    # A score matmul needs its qT and kT at the SAME partition base (matmul
    # constraint), so h0 reads base 0 of blk0/blk2 and h1 base 64; blk3 is
    # a DMA-shifted copy of blk1 with the halves swapped so h2 can
    # alternate row-groups per key block and pair with either neighbor.
    q_loc = {0: (0, 0), 1: (64, 0), 2: (0, 1)}
    k_loc = {0: (0, 2), 1: (64, 2), 2: (0, 3)}

    with tile.TileContext(nc) as tc:
        with tc.tile_pool(name="const", bufs=1) as cpool, \
             tc.tile_pool(name="persist", bufs=1) as ppool, \
             tc.tile_pool(name="xqk", bufs=6) as xpool, \
             tc.tile_pool(name="xv", bufs=4) as xvpool, \
             tc.tile_pool(name="pt", bufs=18) as ptpool, \
             tc.tile_pool(name="ysb", bufs=4) as ypool, \
             tc.tile_pool(name="rcp", bufs=2) as rcppool, \
             tc.tile_pool(name="rr", bufs=4) as rrpool, \
             tc.tile_pool(name="ot", bufs=3) as otpool, \
             tc.tile_pool(name="pp_proj", bufs=1, space="PSUM") as pp_proj, \
             tc.tile_pool(name="pp_sc", bufs=2, space="PSUM") as pp_sc, \
             tc.tile_pool(name="pp_out", bufs=1, space="PSUM") as pp_out:

            # ---- constants ----
            # two triggers so the transfer rides two DMA rings in parallel
            wqkv_sb = cpool.tile([P, KO, 3 * D_LOCAL], dt_x)
            nc.sync.dma_start(wqkv_sb[:, 0:3, :], wqkv_r[:, 0:3, :])
            nc.sync.dma_start(wqkv_sb[:, 3:6, :], wqkv_r[:, 3:6, :])
            wqk_sb = wqkv_sb[:, :, 0:2 * D_LOCAL]
            wv_sb = wqkv_sb[:, :, 2 * D_LOCAL:3 * D_LOCAL]
            wo_sb = cpool.tile([P, 2, D_MODEL], dt_x)
            cm_sb = cpool.tile([P, P], dt_x)
            # ones row at partition 64 — LHS of the denominator-broadcast
            # matmul (partition-64 base matches the den row's partition).
            # Stored as f32r: the BIR verifier requires f32r matmul inputs
            # to be produced already-rounded (memset can't write f32r, so
            # it is filled by a rounding copy from an fp32 scratch row).
            ones64 = cpool.tile([65, 64], F32R)
            ones_f = cpool.tile([65, 64], F32)

            # ---- persistent activations ----
            qkT_sb = ppool.tile([P, 4, n], dt_acc)
            v_sb = ppool.tile([P, TB, H_LOCAL, 66], dt_acc)
            outT_sb = ppool.tile([P, 2, n], dt_acc)

            # ---- emission helpers (phases interleaved below) ----

            def emit_qkproj_load(t, nsplit=1):
                """Start the q/k input DMAs for chunk t.  nsplit>1 issues
                multiple triggers so the transfer rides several DMA rings
                in parallel (a single ring moves only ~100 GB/s)."""
                tok = t * NT
                xq = xpool.tile([P, KO, NT], dt_x, tag="x")
                xk = xpool.tile([P, KO, NT], dt_x, tag="x")
                for s in range(nsplit):
                    k0, k1 = s * KO // nsplit, (s + 1) * KO // nsplit
                    nc.sync.dma_start(xq[:, k0:k1, :],
                                      qt_r[:, k0:k1, tok:tok + NT])
                    nc.sync.dma_start(xk[:, k0:k1, :],
                                      kt_r[:, k0:k1, tok:tok + NT])
                return xq, xk

            def emit_qkproj_blk(t, blk, xq, xk):
                """Project one 128-row M-block of q/k for chunk t.

                The PSUM->SBUF cast runs on the Scalar engine for the
                early chunks (it idles there while the exp pipeline
                fills) and on Vector afterwards."""
                tok = t * NT
                ps = pp_proj.tile([P, NQ], F32, tag="psproj")
                for ko in range(KO):
                    # blk1 contracts q2 against Q-input and k2 against
                    # K-input: split into two half-partition matmuls.
                    if blk == 1:
                        nc.tensor.matmul(
                            ps[0:64, 0:NT],
                            _mm(wqk_sb[:, ko, 128:192], mm),
                            _mm(xq[:, ko, :], mm),
                            start=(ko == 0), stop=(ko == KO - 1),
                            skip_group_check=True,
                        )
                        nc.tensor.matmul(
                            ps[64:128, 0:NT],
                            _mm(wqk_sb[:, ko, 192:256], mm),
                            _mm(xk[:, ko, :], mm),
                            start=(ko == 0), stop=(ko == KO - 1),
                            skip_group_check=True,
                        )
                    else:
                        x = xq if blk == 0 else xk
                        nc.tensor.matmul(
                            ps[:, 0:NT],
                            _mm(wqk_sb[:, ko, blk * 128:(blk + 1) * 128],
                                mm),
                            _mm(x[:, ko, :], mm),
                            start=(ko == 0), stop=(ko == KO - 1),
                        )
                if t <= 3:
                    nc.scalar.copy(qkT_sb[:, blk, tok:tok + NT],
                                   ps[:, 0:NT])
                else:
                    nc.vector.tensor_copy(
                        out=qkT_sb[:, blk, tok:tok + NT],
                        in_=ps[:, 0:NT],
                    )

            def emit_qkproj_shift(t):
                # Partition-shifted copies so h2's scores matmul sees qT/kT
                # at the same base — and at BOTH bases, so h2 can alternate
                # row-groups and pair with whichever half is free:
                #   blk3[0:64]   = k2 (from blk1[64:128])
                #   blk3[64:128] = q2 (from blk1[0:64])
                tok = t * NT
                nc.sync.dma_start(
                    qkT_sb[0:64, 3, tok:tok + NT],
                    qkT_sb[64:128, 1, tok:tok + NT],
                )
                nc.sync.dma_start(
                    qkT_sb[64:128, 3, tok:tok + NT],
                    qkT_sb[0:64, 1, tok:tok + NT],
                )

            def emit_vproj_group(tb0, ntb):
                """Project v for token blocks [tb0, tb0+ntb) with a single
                input DMA (DMA triggers serialize on the Sync queue)."""
                xv = xvpool.tile([P, KO, KB_PER_Q * P], dt_x)
                nc.sync.dma_start(xv[:, :, 0:ntb * P],
                                  vt_r[:, :, tb0 * P:(tb0 + ntb) * P])
                for i in range(ntb):
                    ps = pp_proj.tile([P, NQ], F32, tag="psproj")
                    for ko in range(KO):
                        nc.tensor.matmul(
                            ps[:, 0:D_LOCAL],
                            _mm(xv[:, ko, i * P:(i + 1) * P], mm),
                            _mm(wv_sb[:, ko, :], mm),
                            start=(ko == 0), stop=(ko == KO - 1),
                        )
                    # free sizes match (192); AP element order is h-major
                    # on both sides so a single strided copy works
                    nc.vector.tensor_copy(
                        out=v_sb[:, tb0 + i, :, 0:64],
                        in_=ps[:, 0:D_LOCAL],
                    )

            def emit_outproj(tb, tail=False):
                """y[tb*128:(tb+1)*128, :] = outT[:, tb-block].T @ w_o.

                In the epilogue the attention PSUM banks are free, so use
                the (2-bank) score pool for double-buffering instead of
                serializing on the single projection bank.  Both column
                chunks land in one SBUF tile -> single y DMA per block."""
                ysb = ypool.tile([P, D_MODEL], dt_acc)
                for oc in range(NOC):
                    if tail:
                        ps = pp_sc.tile([P, 2, NQ], F32, tag="psc",
                                        name="psc")[:, 0, :]
                    else:
                        ps = pp_proj.tile([P, NQ], F32, tag="psproj")
                    nc.tensor.matmul(
                        ps[:, 0:NO],
                        _mm(outT_sb[:, 0, tb * P:(tb + 1) * P], mm),
                        _mm(wo_sb[:, 0, oc * NO:(oc + 1) * NO], mm),
                        start=True, stop=False,
                    )
                    nc.tensor.matmul(
                        ps[:, 0:NO],
                        _mm(outT_sb[0:64, 1, tb * P:(tb + 1) * P], mm),
                        _mm(wo_sb[0:64, 1, oc * NO:(oc + 1) * NO], mm),
                        start=False, stop=True,
                    )
                    nc.vector.tensor_copy(out=ysb[:, oc * NO:(oc + 1) * NO],
                                          in_=ps[:, 0:NO])
                nc.sync.dma_start(y_d.ap()[tb * P:(tb + 1) * P, :], ysb[:])

            # Heads are interleaved so the PE runs two concurrent score
            # matmuls on disjoint row-groups: h0 lives at partitions 0-63,
            # h1 at 64-127, h2 alternates base per key-block (its qT/kT are
            # replicated at both bases in blk1/blk3).
            def h2_qk(kb):
                if kb % 2 == 0:
                    return (0, 1), (0, 3)   # q2 @ blk1[0:64], k2' @ blk3[0:64]
                return (64, 3), (64, 1)     # q2' @ blk3[64:128], k2 @ blk1[64:128]

            def qk_for(h, kb):
                if h == 2:
                    return h2_qk(kb)
                return q_loc[h], k_loc[h]

            def emit_scores_head(j, kb2, h, g):
                """Scores + exp + mask for head h, key blocks kb2, kb2+1.

                Returns the (bf16-viewed) P tile and per-block AV offsets."""
                psc2 = pp_sc.tile([P, 2, NQ], F32, tag="psc", name="psc")
                # Both blocks' matmuls start at the PAIR's minimum offset so
                # the wide exp below reads no uninitialized PSUM; the extra
                # columns of the right (diagonal) block are never read by
                # its AV matmul.
                off0 = max(kb2 - KB_PER_Q * j, 0) * P
                offs = {}
                for i, kb in ((0, kb2), (1, kb2 + 1)):
                    (qp, qb), (kp, kb_) = qk_for(h, kb)
                    kloc = kb - KB_PER_Q * j
                    offs[i] = max(kloc, 0) * P  # mask/AV offset
                    nc.tensor.matmul(
                        psc2[:, i, off0:],
                        _mm(qkT_sb[kp:kp + 64, kb_, kb * P:(kb + 1) * P],
                            mm),
                        _mm(qkT_sb[qp:qp + 64, qb,
                                   j * NQ + off0:(j + 1) * NQ], mm),
                        start=True, stop=True,
                    )
                # Exp-engine policy: the Scalar engine alone is the
                # bottleneck (1 elem/cycle at 1.2GHz), so route a growing
                # share of tiles to the Vector engine as the chunks get
                # exp-dense: h2 mostly always, h1 partially in the last
                # chunks (where per-group exp demand exceeds 2 ACTs).
                if h == H_LOCAL - 1:
                    use_dve = (dt_pt == BF16 and j >= dve_min_j
                               and (g % 8) < (7 if j >= 4 else dve8))
                elif h == 1:
                    use_dve = dt_pt == BF16 and j >= QCH - 1 and (g % 8) < 2
                else:
                    use_dve = False
                if use_dve:
                    # One Vector op: int16(A*x+B)'s bit pattern IS the bf16
                    # of ~exp(x); the tile feeds the AV matmul directly.
                    pti = ptpool.tile([P, 2, NQ], I16, name="pt")
                    nc.vector.tensor_scalar(
                        out=pti[:, :, off0:], in0=psc2[:, :, off0:],
                        scalar1=FEXP16_A, scalar2=FEXP16_B,
                        op0=mybir.AluOpType.mult,
                        op1=mybir.AluOpType.add)
                    pt2 = pti.bitcast(BF16)
                else:
                    pt2 = ptpool.tile([P, 2, NQ], dt_pt, name="pt")
                    # One wide exp over both key blocks (2 PSUM banks).
                    nc.scalar.activation(pt2[:, :, off0:],
                                         psc2[:, :, off0:],
                                         mybir.ActivationFunctionType.Exp)
                for i, kb in ((0, kb2), (1, kb2 + 1)):
                    kloc = kb - KB_PER_Q * j
                    if kloc >= 0:
                        off = offs[i]
                        # NOTE: keep GpSimd dedicated to partition_broadcast
                        # — any other op family there triggers a ~7us Q7
                        # library swap per switch.
                        nc.vector.tensor_mul(out=pt2[:, i, off:off + P],
                                             in0=pt2[:, i, off:off + P],
                                             in1=cm_sb[:])
                return pt2, offs

            def emit_scores_group(j, kb2, g, heads=(0, 1, 2)):
                pt2s = {}
                offs = {}
                for h in heads:
                    pt2s[h], offs[h] = emit_scores_head(j, kb2, h, g)
                return pt2s, offs

            def emit_av_group(j, po, kb2, nkb, pt2s, offs):
                """AV accumulation for key blocks kb2, kb2+1 (runs `depth`
                groups behind the scores so the exp engines always have
                the next groups' scores queued ahead of AV work)."""
                for h in range(H_LOCAL):
                    pt2 = pt2s[h]
                    for i, kb in ((0, kb2), (1, kb2 + 1)):
                        off = offs[h][i]
                        nc.tensor.matmul(
                            po[h][0:65, off:],
                            _mm(v_sb[:, kb, h, 0:65], mm),
                            _mm(pt2[:, i, off:], mm),
                            start=(kb == 0), stop=(kb == nkb - 1),
                        )

            def emit_normalize(j, po, c0=0, c1=NQ, tail=False):
                """Divide outT_aug rows by the denominator row for chunk
                j's query columns [c0, c1).

                The muls read the PSUM accumulators directly (no staging
                copy).  The denominator row (partition 64) is staged to
                SBUF with a partition-locked copy, then BROADCAST to
                partitions 0-63 by a 1-partition-contraction PE matmul
                (ones[64]^T x den-row, fp32r so the mantissa survives) —
                no partition-move DMA and no GpSimd in the chain, both of
                which cost microseconds of latency per hop."""
                w = c1 - c0
                q0 = j * NQ + c0
                # h1 first: its outT write has an extra DMA partition-shift
                # hop, and the output projection's first matmul needs both
                # h0 and h1 rows.
                for h in (1, 0, 2):
                    den64 = rcppool.tile([65, NQ], F32R, tag="den64",
                                         name="den64")
                    nc.vector.tensor_copy(out=den64[64:65, c0:c1],
                                          in_=po[h][64:65, c0:c1])
                    rr = rrpool.tile([64, NQ], F32, tag="rr", name="rr")
                    if tail:
                        # Tail: the output projections run from the score
                        # pool, so the projection bank is free — the PE
                        # broadcast is ~3us lower latency than the DMA hop.
                        psn = pp_proj.tile([P, NQ], F32, tag="psproj",
                                           name="psn")
                        nc.tensor.matmul(
                            psn[0:64, c0:c1],
                            ones64[64:65, 0:64],
                            den64[64:65, c0:c1],
                            start=True, stop=True,
                        )
                        nc.vector.reciprocal_approx_fast(
                            out=rr[:, 0:w], in_=psn[0:64, c0:c1])
                    else:
                        # Mid-run: the projection bank is oversubscribed
                        # (qk/v/out projections all rotate through it), so
                        # move the denominator row with a DMA instead; its
                        # ~2us latency is off the critical path here.
                        den0 = rcppool.tile([1, NQ], F32, tag="den0",
                                            name="den0")
                        nc.sync.dma_start(den0[:, 0:w],
                                          den64.bitcast(F32)[64:65, c0:c1])
                        rcp = rcppool.tile([1, NQ], F32, tag="rcp",
                                           name="rcp")
                        nc.vector.reciprocal_approx_fast(out=rcp[:, 0:w],
                                                         in_=den0[:, 0:w])
                        nc.gpsimd.partition_broadcast(rr[:, 0:w],
                                                      rcp[:, 0:w],
                                                      channels=64)
                    if h == 1:
                        # h1 lives at partitions 64-127 of outT blk0; DVE
                        # lanes are partition-locked, so write a temp at
                        # base 0 and DMA partition-shift it up.
                        ot = otpool.tile([64, NQ], dt_acc, name="ot")
                        nc.vector.tensor_mul(out=ot[:, 0:w],
                                             in0=po[h][0:64, c0:c1],
                                             in1=rr[:, 0:w])
                        nc.sync.dma_start(
                            outT_sb[64:128, 0, q0:q0 + w], ot[:, 0:w])
                    else:
                        dst = outT_sb[0:64, 0 if h == 0 else 1, q0:q0 + w]
                        nc.vector.tensor_mul(out=dst,
                                             in0=po[h][0:64, c0:c1],
                                             in1=rr[:, 0:w])

            # ---- software-pipelined emission ----
            # Global pending-AV queue: (j, po, nkb, kb2, pt2s, offs).
            pend = []
            next_tb = 0
            normalized = [0]  # chunks whose outT is fully normalized

            def pop_one(last_chunk_norm=True):
                jj, po_, nkb_, kb2_, pt2s, offs = pend.pop(0)
                emit_av_group(jj, po_, kb2_, nkb_, pt2s, offs)
                if kb2_ == nkb_ - 2 and (last_chunk_norm or jj < QCH - 1):
                    emit_normalize(jj, po_)
                    normalized[0] = jj + 1

            # Output projections are deferred to the late chunks, where the
            # exp engines are saturated and the PE has slack; in the early
            # chunks the PE is the bottleneck.
            op_start = max(QCH - op_back, 1)

            # Prologue.  The critical path to the FIRST exp is:
            # wqkv + x-chunk-0 DMAs (split across rings), the blk0 (q01)
            # and blk2 (k01) projections, one CAST each, h0's two score
            # matmuls.  A chain of dummy matmuls on a zeroed tile warms the
            # PE clock (the PE ramps to full speed only after ~3us of
            # continuous work) while the input DMAs are still in flight.
            warm = cpool.tile([P, NQ], dt_x)
            nc.vector.memset(warm[:], 0.0)
            nc.vector.memset(v_sb[:, :, :, 64:65], 1.0)
            nc.vector.memset(ones_f[64:65, :], 1.0)
            nc.vector.tensor_copy(out=ones64[64:65, :],
                                  in_=ones_f[64:65, :])
            xq0, xk0 = emit_qkproj_load(0, nsplit=2)
            psw = pp_out.tile([P, NQ], F32, tag="po0", name="warmps")
            for r in range(10):
                nc.tensor.matmul(psw[:, 0:NQ], _mm(warm[:, 0:128], mm),
                                 _mm(warm[:], mm),
                                 start=(r == 0), stop=(r == 9))
            emit_qkproj_blk(0, 0, xq0, xk0)
            nc.sync.dma_start(cm_sb[:], cm_d.ap())
            emit_qkproj_blk(0, 2, xq0, xk0)
            g0_pt, g0_offs = emit_scores_group(0, 0, 0, heads=(0, 1))
            emit_qkproj_blk(0, 1, xq0, xk0)
            emit_qkproj_shift(0)
            pt_h2, offs_h2 = emit_scores_group(0, 0, 0, heads=(2,))
            g0_pt.update(pt_h2)
            g0_offs.update(offs_h2)

            # Per-chunk projection tasks, dispensed one-or-more per
            # pair-group so the PE never sees a 24-matmul projection clump
            # between two score groups (which starves the exp engines).
            xs = {}

            def make_tasks(j):
                tasks = []
                t = j + 1
                if t < TCH:
                    def load(t=t):
                        xs[t] = emit_qkproj_load(t)
                    tasks.append(load)
                    for blk in (0, 2, 1):
                        def doblk(t=t, blk=blk):
                            emit_qkproj_blk(t, blk, *xs[t])
                            if blk == 1:
                                emit_qkproj_shift(t)
                        tasks.append(doblk)
                if j == 0:
                    tasks.append(lambda: nc.sync.dma_start(
                        wo_sb[:],
                        wo_d.ap().rearrange("p (c m) -> p c m", c=2)))
                    vt0, vt1 = 0, min(2 * KB_PER_Q, TB)
                else:
                    vt0, vt1 = KB_PER_Q * (j + 1), min(KB_PER_Q * (j + 2),
                                                       TB)
                for a in range(vt0, vt1, 2):
                    tasks.append(lambda a=a, b=min(a + 2, vt1):
                                 emit_vproj_group(a, b - a))
                return tasks

            gctr = 1
            for j in range(QCH):
                po = [pp_out.tile([P, NQ], F32, tag=f"po{h}", name=f"po{h}")
                      for h in range(H_LOCAL)]
                nkb = KB_PER_Q * (j + 1)
                npairs = nkb // 2
                tasks = make_tasks(j)
                for p in range(npairs):
                    if j == 0 and p == 0:
                        # scores/exp emitted in the prologue
                        pend.append((0, po, nkb, 0, g0_pt, g0_offs))
                    else:
                        pt2s, offs = emit_scores_group(j, 2 * p, gctr)
                        gctr += 1
                        pend.append((j, po, nkb, 2 * p, pt2s, offs))
                    # Interleave next-chunk projections and deferred output
                    # projections between attention groups.  Tasks dispense
                    # BEFORE the AV pops: a v-projection must be emitted
                    # ahead of the first AV group that reads it.
                    if tasks:
                        k = -(-len(tasks) // (npairs - p))
                        if j == 0 and p == 0:
                            # chunk-1's projection blocks would sit stalled
                            # (waiting on their input DMA) AHEAD of chunk
                            # 0's remaining scores in the in-order PE
                            # queue; emit only the DMA-trigger task here.
                            k = 1
                        for _ in range(k):
                            tasks.pop(0)()
                    elif (j >= op_start and p >= 2 and p % 2 == 0
                          and p <= npairs - 2):
                        for _ in range(op_batch):
                            if next_tb < KB_PER_Q * normalized[0]:
                                emit_outproj(next_tb)
                                next_tb += 1
                    # Eagerly drain the queue near the end of the final
                    # chunk so only the last AV group remains for the tail.
                    limit = depth
                    if j == QCH - 1:
                        limit = min(depth, max(1, npairs - 1 - p))
                    while len(pend) > limit:
                        pop_one(last_chunk_norm=False)

            # Epilogue: drain the pending queue; the final chunk's last AV
            # group is followed by a SPLIT normalize so the last output
            # projections overlap the second half's normalize chain.
            while len(pend) > 1:
                pop_one(last_chunk_norm=False)
            jj, po_, nkb_, kb2_, pt2s, offs = pend.pop(0)
            emit_av_group(jj, po_, kb2_, nkb_, pt2s, offs)
            tb_last = KB_PER_Q * jj
            for tb in range(next_tb, tb_last):
                emit_outproj(tb, tail=True)
            # Normalize the final chunk in 128-query quarters, each
            # immediately followed by its output projection, so the last
            # projections overlap the remaining normalize chains.
            for qr in range(KB_PER_Q):
                emit_normalize(jj, po_, qr * P, (qr + 1) * P, tail=True)
                if tb_last + qr < TB:
                    emit_outproj(tb_last + qr, tail=True)

    nc.compile()
    return nc


def make_causal_mask_np(dt=np.float32):
    """[128, 128] lower-left keep mask: m[p, f] = 1.0 iff f >= p."""
    f = np.arange(P)[None, :]
    p = np.arange(P)[:, None]
    return (f >= p).astype(np.float32).astype(dt)


def prep_core_inputs(Q, K, V, w_q, w_k, w_v, w_o, core, n=N_TOKENS,
                     np_x=ml_dtypes.bfloat16, np_pt=ml_dtypes.bfloat16):
    """Host-side sharding/layout prep for one core. All fp32 numpy in."""
    b = core // 4
    g = core % 4
    hs = g * D_LOCAL
    scale = 1.0 / np.sqrt(D_K)
    qt = np.ascontiguousarray(Q[b].T).astype(np_x)
    kt = np.ascontiguousarray(K[b].T).astype(np_x)
    vt = np.ascontiguousarray(V[b].T).astype(np_x)
    wql = w_q[hs:hs + D_LOCAL] * scale
    wkl = w_k[hs:hs + D_LOCAL]
    # column order [q0 q1 | q2 k2 | k0 k1 | v] (see build_nc); q/k/v
    # packed into one tensor so the device loads them with few DMAs
    wqkv = np.ascontiguousarray(
        np.concatenate([wql[0:128], wql[128:192], wkl[128:192], wkl[0:128],
                        w_v[hs:hs + D_LOCAL]], axis=0).T
    ).astype(np_x)
    # w_o row-chunks ([0:128] then [128:192] at partitions 0-63)
    wo = w_o[:, hs:hs + D_LOCAL].T
    wo_pack = np.zeros((P, 2 * D_MODEL), dtype=np.float32)
    wo_pack[:, 0:D_MODEL] = wo[0:P]
    wo_pack[0:64, D_MODEL:2 * D_MODEL] = wo[P:D_LOCAL]
    return {"qt": qt, "kt": kt, "vt": vt, "wqkv": wqkv,
            "wo": wo_pack.astype(np_x),
            "cm": make_causal_mask_np(np_x)}


_NC_CACHE = {}


def _get_nc(key, **kw):
    if key not in _NC_CACHE:
        _NC_CACHE[key] = build_nc(**kw)
    return _NC_CACHE[key]


KCFG = {"mm": "bf16", "dt_x": BF16, "dt_pt": BF16, "dt_acc": BF16,
        "np_x": ml_dtypes.bfloat16, "np_pt": ml_dtypes.bfloat16}


def kernel(Q, K, V, w_q, w_k, w_v, w_o):
    Q = np.asarray(Q, dtype=np.float32)
    K = np.asarray(K, dtype=np.float32)
    V = np.asarray(V, dtype=np.float32)
    w_q = np.asarray(w_q, dtype=np.float32)
    w_k = np.asarray(w_k, dtype=np.float32)
    w_v = np.asarray(w_v, dtype=np.float32)
    w_o = np.asarray(w_o, dtype=np.float32)

    nc = _get_nc((KCFG["mm"], str(KCFG["dt_x"])),
                 n=N_TOKENS, mm=KCFG["mm"], dt_x=KCFG["dt_x"],
                 dt_pt=KCFG["dt_pt"], dt_acc=KCFG["dt_acc"])
    in_maps = [
        prep_core_inputs(Q, K, V, w_q, w_k, w_v, w_o, c,
                         np_x=KCFG["np_x"], np_pt=KCFG["np_pt"])
        for c in range(N_CORES)
    ]
    res = bass_utils.run_bass_kernel_spmd(nc, in_maps,
                                          core_ids=list(range(N_CORES)))
    out = np.zeros((B, N_TOKENS, D_MODEL), dtype=np.float32)
    for c in range(N_CORES):
        out[c // 4] += np.asarray(res.results[c]["y"], dtype=np.float32)
    return out
